# revision 13
# baseline (speedup 1.0000x reference)
"""Trainium2 Bass kernel for nn_ChunkedAttention (== full multi-head attention).

Math (reference):
    q = x @ wq.T + bq ; k,v likewise            x: [B=2, S=2048, D=1024]
    per head h (Dk=64): attn = softmax(q k^T / 8) ; o_h = attn @ v_h
    out = concat_h(o_h) @ wo.T + bo

Sharding (8 cores): head/tensor parallel. Core c owns heads {2c, 2c+1} for
BOTH batches, i.e. feature rows F = [128c, 128c+128) of wq/wk/wv and columns
F of wo.  Host pre-transposes x -> xT [D, T] (T = B*S = 4096) and the weight
slices; each core computes the partial row-parallel O-projection
outT_c = (attn_pair @ wo[:, F].T).T  in [D, T] layout; host sums the 8
partials, transposes, and adds bo.  No collectives -> all device time is
compute.

Per-core device pipeline (everything f32 data, matmuls in float32r which
streams at 1 cycle/row for moving dim >= 256):
  phase 1 (per 512-col t-tile): qT/kT/vT = wT.T @ xT  (K=8x128, N=512)
     PSUM evacuated by ScalarE with the per-partition bias.  vT is
     transposed back to natural [t, d] layout on the PE (128x128 tiles) and
     stored with TWO ones-columns between the heads: vn = [v_h0 | 1 | 1 | v_h1].
  phase 2 (per batch, head, 512-wide q-tile): scoresT_j = kT_j.T @ qT
     ([128 k-part, 512 q] in PSUM, K=64), exp on ScalarE in [128, 1024]
     batches straight out of PSUM, then oT += v_ext_j.T @ expT_j where
     v_ext = vn[:, 0:65] (h0, denom lands in row 64) or vn[:, 65:130]
     (h1, written at partition offset 63 so the denom lands in row 63 and
     data in rows 64..127 -- keeps every later op lane-aligned).
     Normalize: DVE reciprocal of the denom row, PE broadcasts it across 64
     partitions (ones[1,64].T @ recip[1,512]), DVE multiplies -> opair
     [128 = h0 d | h1 d, 512].
  phase 3: finalT_m = woT_m.T @ opair (K=128), DVE evac, DMA out.

t-tiles are emitted batch-0-first so the Tile scheduler overlaps batch-0
attention with batch-1 projections.
"""

import numpy as np

import concourse.bass as bass
import concourse.bacc as bacc
import concourse.tile as tile
from concourse import mybir
from concourse.bass_utils import run_bass_kernel_spmd

F32 = mybir.dt.float32
F32R = mybir.dt.float32r
BF16 = mybir.dt.bfloat16
AF = mybir.ActivationFunctionType
MMDT = F32R          # dtype for matmul operands (F32R or BF16)

D_MODEL = 1024
B = 2
N_CORES = 8
P = 128




def build_program(S=2048):
    T = B * S
    TT = min(512, S)        # free-dim tile size (q-tile / t-tile)
    NT = T // TT            # t-tiles over both batches
    KC = D_MODEL // P       # contraction tiles for projections
    NJ = S // P             # key tiles per batch
    NQ = S // TT            # q tiles per batch
    NM = D_MODEL // P       # output-feature tiles
    JPT = TT // P           # 128-row blocks per t-tile
    GJ = max(1, min(NJ, 1024 // TT))  # j-tiles per exp batch
    NG = NJ // GJ

    nc = bacc.Bacc()
    xT = nc.dram_tensor("xT", [D_MODEL, T], MMDT, kind="ExternalInput")
    wqT = nc.dram_tensor("wqT", [D_MODEL, P], MMDT, kind="ExternalInput")
    wkT = nc.dram_tensor("wkT", [D_MODEL, P], MMDT, kind="ExternalInput")
    wvT = nc.dram_tensor("wvT", [D_MODEL, P], MMDT, kind="ExternalInput")
    woT = nc.dram_tensor("woT", [P, D_MODEL], MMDT, kind="ExternalInput")
    bqkv = nc.dram_tensor("bqkv", [P, 3], F32, kind="ExternalInput")
    identD = nc.dram_tensor("identD", [P, P], MMDT, kind="ExternalInput")
    onesD = nc.dram_tensor("onesD", [P, 64], MMDT, kind="ExternalInput")
    outT = nc.dram_tensor("outT", [D_MODEL, T], F32, kind="ExternalOutput")

    with tile.TileContext(nc) as tc:
        with (
            nc.allow_low_precision(
                reason="f32r tiles everywhere; matmul accumulation is f32"),
            tc.tile_pool(name="consts", bufs=1) as consts,
            tc.tile_pool(name="resident", bufs=1) as resident,
            tc.tile_pool(name="xin", bufs=2 * KC) as xin,
            tc.tile_pool(name="vtmp", bufs=2) as vtmp,
            tc.tile_pool(name="expool", bufs=2) as expool,
            tc.tile_pool(name="small", bufs=2) as small,
            tc.tile_pool(name="oo", bufs=3) as oo,
            # PSUM: 8 banks total.  big: 2x[128,1024] = 4 banks (proj qkv +
            # scores).  psO: 2x[128,512] = 2 banks (attn-out accumulators).
            # psX: 2x[128,512] = 2 banks (v-transpose, recip-bcast, final).
            tc.tile_pool(name="psA", bufs=2, space="PSUM") as psA,
            tc.tile_pool(name="psO", bufs=2, space="PSUM") as psO,
            tc.tile_pool(name="psX", bufs=2, space="PSUM") as psX,
        ):
            ident = consts.tile([P, P], MMDT)
            nc.sync.dma_start(out=ident, in_=identD[:, :])
            ones_t = consts.tile([P, 64], MMDT)
            nc.sync.dma_start(out=ones_t, in_=onesD[:, :])
            b3 = consts.tile([P, 3], F32)
            nc.sync.dma_start(out=b3, in_=bqkv[:, :])

            w_tiles = {}
            for name, dram in (("q", wqT), ("k", wkT), ("v", wvT)):
                tl = []
                for kk in range(KC):
                    t_ = consts.tile([P, P], MMDT, tag=f"w{name}{kk}")
                    nc.sync.dma_start(out=t_, in_=dram[P * kk:P * (kk + 1), :])
                    tl.append(t_)
                w_tiles[name] = tl
            wo_tiles = []
            for m in range(NM):
                t_ = consts.tile([P, P], MMDT, tag=f"wo{m}")
                nc.sync.dma_start(out=t_, in_=woT[:, P * m:P * (m + 1)])
                wo_tiles.append(t_)

            qT_sb = resident.tile([P, T], MMDT)
            kT_sb = resident.tile([P, T], MMDT)
            vn = [resident.tile([P, 130], MMDT, tag=f"vn{jt}", name=f"vn{jt}")
                  for jt in range(T // P)]

            # ---- phase 1: projections ----
            for tt in range(NT):
                xts = []
                for kk in range(KC):
                    xt_ = xin.tile([P, TT], MMDT, tag="xt")
                    nc.sync.dma_start(
                        out=xt_, in_=xT[P * kk:P * (kk + 1), TT * tt:TT * (tt + 1)])
                    xts.append(xt_)
                for name, bcol in (("q", 0), ("k", 1), ("v", 2)):
                    ps = psA.tile([P, TT], F32, tag="big")
                    wt = w_tiles[name]
                    for kk in range(KC):
                        nc.tensor.matmul(ps, (wt[kk]), (xts[kk]),
                                         start=(kk == 0), stop=(kk == KC - 1))
                    bias = b3[:, bcol:bcol + 1]
                    if name == "q":
                        nc.scalar.activation(
                            out=qT_sb[:, TT * tt:TT * (tt + 1)], in_=ps,
                            func=AF.Identity, bias=bias, scale=1.0)
                    elif name == "k":
                        nc.scalar.activation(
                            out=kT_sb[:, TT * tt:TT * (tt + 1)], in_=ps,
                            func=AF.Identity, bias=bias, scale=1.0)
                    else:
                        vtt = vtmp.tile([P, TT], MMDT, tag="vtt")
                        nc.scalar.activation(out=vtt, in_=ps, func=AF.Identity,
                                             bias=bias, scale=1.0)
                        for jj in range(JPT):
                            jt = tt * JPT + jj
                            tp = psX.tile([P, P], MMDT, tag="aux")
                            nc.tensor.transpose(
                                tp, vtt[:, P * jj:P * (jj + 1)], ident)
                            # vn = [v_h0 | 1 | v_h1 | 1]: each head's 65-col
                            # slice vn[:, 65h:65h+65] has its denom col last.
                            nc.sync.dma_start(out=vn[jt][:, 64:65],
                                              in_=onesD[:, 0:1])
                            nc.sync.dma_start(out=vn[jt][:, 129:130],
                                              in_=onesD[:, 0:1])
                            nc.vector.tensor_copy(vn[jt][:, 0:64], tp[:, 0:64])
                            nc.vector.tensor_copy(vn[jt][:, 65:129], tp[:, 64:128])

            # ---- phases 2+3: attention + O-projection ----
            for b_ in range(B):
                tb = b_ * S
                for qt in range(NQ):
                    q0 = tb + TT * qt
                    opair = oo.tile([P, TT], MMDT, tag="opair")
                    for h in range(2):
                        q_ap = qT_sb[64 * h:64 * (h + 1), q0:q0 + TT]
                        ot = psO.tile([P, TT], F32, tag="ot")
                        for g in range(NG):
                            sc = psA.tile([P, GJ * TT], F32, tag="big")
                            for u in range(GJ):
                                j = g * GJ + u
                                k_ap = kT_sb[64 * h:64 * (h + 1),
                                             tb + P * j:tb + P * (j + 1)]
                                nc.tensor.matmul(
                                    sc[:, TT * u:TT * (u + 1)], (k_ap), (q_ap),
                                    start=True, stop=True, skip_group_check=True)
                            ex = expool.tile([P, GJ * TT], MMDT, tag="ex")
                            nc.scalar.activation(out=ex, in_=sc, func=AF.Exp,
                                                 scale=0.125)
                            for u in range(GJ):
                                j = g * GJ + u
                                jt = (tb // P) + j
                                vsl = vn[jt][:, 65 * h:65 * h + 65]
                                nc.tensor.matmul(
                                    ot[0:65, :], (vsl),
                                    (ex[:, TT * u:TT * (u + 1)]),
                                    start=(j == 0), stop=(j == NJ - 1),
                                    skip_group_check=True)
                        # normalize this head's rows into opair.  The h==1
                        # write shifts quadrants (in parts 0-63 -> out 64-127)
                        # which DVE supports at nch<=64.
                        rr = small.tile([P, TT], MMDT, tag="rr")
                        nc.vector.reciprocal(out=rr[64:65, :], in_=ot[64:65, :])
                        rb = psX.tile([P, TT], F32, tag="aux")
                        nc.tensor.matmul(rb[0:64, :], (ones_t[64:65, :]),
                                         (rr[64:65, :]),
                                         start=True, stop=True,
                                         skip_group_check=True)
                        rbs = small.tile([P, TT], F32, tag="rbs")
                        nc.vector.tensor_copy(rbs[0:64, :], rb[0:64, :])
                        nc.vector.tensor_mul(opair[64 * h:64 * h + 64, :],
                                             ot[0:64, :], rbs[0:64, :])
                    for m in range(NM):
                        fin = psX.tile([P, TT], F32, tag="aux")
                        nc.tensor.matmul(fin, (wo_tiles[m]), (opair),
                                         start=True, stop=True,
                                         skip_group_check=True)
                        fsb = oo.tile([P, TT], F32, tag="fsb")
                        nc.vector.tensor_copy(fsb, fin)
                        nc.sync.dma_start(
                            out=outT[P * m:P * (m + 1), q0:q0 + TT], in_=fsb)
    nc.compile()
    return nc


_CACHE = {}


def get_program(S=2048):
    if S not in _CACHE:
        _CACHE[S] = build_program(S)
    return _CACHE[S]


def make_in_maps(x, wq, bq, wk, bk, wv, bv, wo):
    np_mm = mybir.dt.np(MMDT)
    Bv, Sv, Dv = x.shape
    xT = np.ascontiguousarray(x.reshape(Bv * Sv, Dv).T).astype(np_mm)
    in_maps = []
    for c in range(N_CORES):
        F = slice(P * c, P * (c + 1))
        in_maps.append({
            "xT": xT,
            "wqT": np.ascontiguousarray(wq[F, :].T).astype(np_mm),
            "wkT": np.ascontiguousarray(wk[F, :].T).astype(np_mm),
            "wvT": np.ascontiguousarray(wv[F, :].T).astype(np_mm),
            "woT": np.ascontiguousarray(wo[:, F].T).astype(np_mm),
            "bqkv": np.ascontiguousarray(
                np.stack([bq[F], bk[F], bv[F]], axis=1).astype(np.float32)),
            "identD": np.eye(P, dtype=np_mm),
            "onesD": np.ones((P, 64), np_mm),
        })
    return in_maps


def kernel(x, wq, bq, wk, bk, wv, bv, wo, bo, _trace=False, _trace_cores=None):
    x = np.asarray(x, np.float32)
    wq = np.asarray(wq, np.float32)
    wk = np.asarray(wk, np.float32)
    wv = np.asarray(wv, np.float32)
    wo = np.asarray(wo, np.float32)
    bq = np.asarray(bq, np.float32)
    bk = np.asarray(bk, np.float32)
    bv = np.asarray(bv, np.float32)
    bo = np.asarray(bo, np.float32)
    Bv, Sv, Dv = x.shape
    nc = get_program(Sv)
    in_maps = make_in_maps(x, wq, bq, wk, bk, wv, bv, wo)
    res = run_bass_kernel_spmd(nc, in_maps, core_ids=list(range(N_CORES)),
                               trace=_trace, trace_cores=_trace_cores)
    acc = res.results[0]["outT"].astype(np.float32)
    for r in res.results[1:]:
        acc = acc + r["outT"].astype(np.float32)
    out = acc.T + bo[None, :]
    if _trace:
        kernel.last_results = res
    return np.ascontiguousarray(out.reshape(Bv, Sv, Dv).astype(np.float32))
